# revision 1
# baseline (speedup 1.0000x reference)
"""Bass TRN2 kernel for the boundary cosine-similarity context loss.

Per core (8 cores): batch b = k//2, row-half h = k%2; slab = 194 rows
(h==0: global rows 0..193, h==1: 190..383); produced = slab rows 2..191.
19 macroblocks x 10 produced rows.

Per macroblock n (y0 = 2+10n):
  g window  : bf16 features, slab rows y0-2..y0+11 (14 rows) + pad
  norms     : fresh-row squares -> ones-col matmuls -> PSUM n2 -> recip ->
              sqrt -> inv12 sliding tile [12, W+4] (rows y0..y0+11)
  dots      : 2 waves x 12 shifts: DVE bf16 mult t = g * g_shift,
              ones-col matmuls reduce C -> PSUM [12, W] per produced row
  pack      : ACT copy psum->bf16 s_r, DMA into pk [120, W] (row 12r+m)
  IP/IQ     : selection matmuls from inv12 -> PSUM [120, W] -> ACT copy
  post      : u = (pk*iq*ip - lab); acc[:, n] += sum(ww*u*u)  (DVE TTR)
Host folds the per-batch 1/cnt, 1/24, valid, 1/n_valid scaling.
"""

import numpy as np
import ml_dtypes

import concourse.bass as bass
import concourse.mybir as mybir
from concourse.tile import TileContext
from concourse.vector_clock import ScopedClock
from concourse import bass_utils

BF16 = ml_dtypes.bfloat16
F32 = mybir.dt.float32
BF = mybir.dt.bfloat16

W = 384
C = 128
SHIFTS = [(0, 1), (0, 2),
          (1, -2), (1, -1), (1, 0), (1, 1), (1, 2),
          (2, -2), (2, -1), (2, 0), (2, 1), (2, 2)]
OFFS = [dy * W + dx for dy, dx in SHIFTS]
NSH = 12
RPB = 10
NROWS = NSH * RPB      # 120 packed rows
GW = 14 * W
GPAD = 8


def _patch_tile_drain():
    if getattr(TileContext, "_drain_patched", False):
        return

    def _drain_and_barrier(self, tick_clock, wait_clock):
        drain_inst = self.nc.sync.drain()
        wait_clock.add_sem_waits(
            drain_inst.ins, ScopedClock({None: tick_clock.global_clock}))
        si = drain_inst.ins.sync_info
        if si is not None and si.on_wait and len(si.on_wait) > 1:
            waits = list(si.on_wait)
            drain_inst.ins.sync_info = mybir.SyncInfo(
                on_wait=[waits[-1]], on_update=list(si.on_update or []))
            for w in waits[:-1]:
                nop = self.nc.sync.nop(nofuse=True)
                nop.ins.sync_info = mybir.SyncInfo(on_wait=[w], on_update=[])
        self.nc.all_engine_barrier()
        popped = self.nc._tile_sem_poison_stack.pop()
        assert popped is self._sem_poison
        self.nc.clear_and_free_semaphores(list(self.sems.allocated().values()))
        self.nc.all_engine_barrier()

    TileContext._drain_and_barrier = _drain_and_barrier
    TileContext._drain_patched = True


_WSPLIT_N = [0]


def _split_multi_waits(nc, max_waits=1):
    """This container's walrus rejects instructions with more than one sync
    wait; hoist excess waits onto same-engine NOPs inserted just before."""
    for fn in nc.m.functions:
        for blk in fn.blocks:
            insts = blk.instructions
            out = []
            for inst in insts:
                si = inst.sync_info
                if si is not None and si.on_wait and len(si.on_wait) > max_waits:
                    waits = list(si.on_wait)
                    keep = waits[-max_waits:]
                    for w in waits[:-max_waits]:
                        _WSPLIT_N[0] += 1
                        nop = mybir.InstNoOp(
                            name=f"wsplit_{_WSPLIT_N[0]}", ins=[], outs=[])
                        nop.engine = inst.engine
                        nop.sync_info = mybir.SyncInfo(on_wait=[w],
                                                       on_update=[])
                        out.append(nop)
                    inst.sync_info = mybir.SyncInfo(
                        on_wait=keep, on_update=list(si.on_update or []))
                out.append(inst)
            blk.instructions = out


def build_nc(nblk=19, repeat=1):
    _patch_tile_drain()
    slab_rows = 4 + RPB * nblk
    npix = slab_rows * W

    nc = bass.Bass()
    x = nc.dram_tensor("x", [C, npix], F32, kind="ExternalInput")
    labw = nc.dram_tensor("labw", [nblk, NROWS, 2 * W], BF,
                          kind="ExternalInput")
    eye = nc.dram_tensor("eye", [C, NSH, NSH], BF, kind="ExternalInput")
    eq = nc.dram_tensor("eq", [NSH, 6, NROWS], BF, kind="ExternalInput")
    out = nc.dram_tensor("out", [NROWS, nblk], F32, kind="ExternalOutput")

    with TileContext(nc) as tc:
        with (tc.tile_pool(name="const", bufs=1) as cpool,
              tc.tile_pool(name="gbuf", bufs=2) as gpool,
              tc.tile_pool(name="stage", bufs=2) as spool,
              tc.tile_pool(name="sq", bufs=2) as sqpool,
              tc.tile_pool(name="tm", bufs=8) as tpool,
              tc.tile_pool(name="inv", bufs=2) as ipool,
              tc.tile_pool(name="pack", bufs=3) as packpool,
              tc.tile_pool(name="post", bufs=2) as postpool,
              tc.tile_pool(name="npsum", bufs=1, space="PSUM") as npsum,
              tc.tile_pool(name="dpsum", bufs=6, space="PSUM") as dpsum,
              tc.tile_pool(name="qpsum", bufs=1, space="PSUM") as qpsum):

            eye_sb = cpool.tile([C, NSH, NSH], BF)
            nc.sync.dma_start(eye_sb[:], eye[:])
            eq_sb = cpool.tile([NSH, 6, NROWS], BF)
            nc.sync.dma_start(eq_sb[:], eq[:])
            acc = cpool.tile([NROWS, nblk], F32)
            nc.vector.memset(acc[:], 0.0)

            g_prev = None
            for n in [i for _ in range(repeat) for i in range(nblk)]:
                y0 = 2 + RPB * n
                win0 = (y0 - 2) * W

                # ---- feature window ----
                g = gpool.tile([C, GW + GPAD], BF, tag="g")
                godd = gpool.tile([C, GW + GPAD], BF, tag="godd")
                if n == 0:
                    stage = spool.tile([C, GW], F32, tag="stage")
                    nc.sync.dma_start(stage[:], x[:, 0:GW])
                    nc.gpsimd.tensor_copy(g[:, 0:GW], stage[:])
                    nc.gpsimd.tensor_copy(godd[:, 0:GW - 1], stage[:, 1:GW])
                    nc.gpsimd.memset(godd[:, GW - 1:GW + GPAD], 0.0)
                else:
                    nc.sync.dma_start(g[:, 0:4 * W], g_prev[:, RPB * W:GW])
                    nc.sync.dma_start(godd[:, 0:4 * W - 1],
                                      godd_prev[:, RPB * W:GW - 1])
                    stage = spool.tile([C, RPB * W], F32, tag="stage")
                    nc.sync.dma_start(stage[:], x[:, win0 + 4 * W:win0 + GW])
                    nc.gpsimd.tensor_copy(g[:, 4 * W:GW], stage[:])
                    nc.gpsimd.tensor_copy(godd[:, 4 * W - 1:GW - 1], stage[:])
                    if n <= 1:
                        nc.gpsimd.memset(godd[:, GW - 1:GW + GPAD], 0.0)
                if n <= 1:
                    nc.gpsimd.memset(g[:, GW:GW + GPAD], 0.0)
                fresh0, nfr = 2 * W, 12

                # ---- norms of fresh rows ----
                sqt = sqpool.tile([C, 12 * W], BF, tag="sq")
                nc.scalar.square(sqt[:, 0:nfr * W],
                                 g[:, fresh0:fresh0 + nfr * W])
                n2 = npsum.tile([NSH, W], F32, tag="n2")
                for r in range(nfr):
                    nc.tensor.matmul(n2[:], eye_sb[:, r, :],
                                     sqt[:, r * W:(r + 1) * W],
                                     start=(r == 0), stop=(r == nfr - 1))
                lnt = ipool.tile([NSH, W], F32, tag="lnt")
                nc.scalar.activation(lnt[:], n2[:],
                                     mybir.ActivationFunctionType.Ln)
                inv12 = ipool.tile([NSH, W + 4], BF, tag="inv12")
                nc.scalar.activation(inv12[:, 2:W + 2], lnt[:],
                                     mybir.ActivationFunctionType.Exp,
                                     scale=-0.5)
                if n <= 1:
                    nc.gpsimd.memset(inv12[:, 0:2], 0.0)
                    nc.gpsimd.memset(inv12[:, W + 2:W + 4], 0.0)

                # ---- dots ----
                pk = postpool.tile([NROWS, W], BF, tag="pk")
                s_all = packpool.tile([NSH, RPB * W], BF, tag="sall")
                for w in range(2):
                    pd = [dpsum.tile([NSH, W], F32, tag="pd", name=f"pd{w}_{i}")
                          for i in range(5)]
                    morder = ([m for m in range(NSH) if OFFS[m] % 2 == 0]
                              + [m for m in range(NSH) if OFFS[m] % 2 == 1])
                    for mi, m in enumerate(morder):
                        off = OFFS[m]
                        t = tpool.tile([C, 5 * W], BF, tag="t")
                        base = 2 * W + w * 5 * W
                        in0 = g[:, base:base + 5 * W]
                        if off % 2 == 0:
                            in1 = g[:, base + off:base + off + 5 * W]
                        else:
                            in1 = godd[:, base + off - 1:
                                       base + off - 1 + 5 * W]
                        nc.vector.tensor_mul(t[:], in0, in1)
                        for r5 in range(5):
                            nc.tensor.matmul(pd[r5][:], eye_sb[:, m, :],
                                             t[:, r5 * W:(r5 + 1) * W],
                                             start=(mi == 0),
                                             stop=(mi == NSH - 1))
                    for r5 in range(5):
                        nc.scalar.copy(s_all[:, (w * 5 + r5) * W:
                                             (w * 5 + r5 + 1) * W],
                                       pd[r5][:])

                for r in range(RPB):
                    nc.sync.dma_start(pk[NSH * r:NSH * (r + 1), :],
                                      s_all[:, r * W:(r + 1) * W])

                # ---- IP / IQ via selection matmuls ----
                ipp = qpsum.tile([NROWS, W], F32, tag="iqp")
                nc.tensor.matmul(ipp[:], eq_sb[:, 0, :], inv12[:, 2:W + 2],
                                 start=True, stop=True)
                ip = postpool.tile([NROWS, W], BF, tag="ip")
                nc.scalar.copy(ip[:], ipp[:])

                iqp = qpsum.tile([NROWS, W], F32, tag="iqp")
                for di in range(5):
                    dx = di - 2
                    nc.tensor.matmul(iqp[:], eq_sb[:, 1 + di, :],
                                     inv12[:, 2 + dx:2 + dx + W],
                                     start=(di == 0), stop=(di == 4))
                iq = postpool.tile([NROWS, W], BF, tag="iq")
                nc.scalar.copy(iq[:], iqp[:])

                lw = postpool.tile([NROWS, 2 * W], BF, tag="lw")
                nc.sync.dma_start(lw[:], labw[n])
                lab = lw[:, 0:W]
                ww = lw[:, W:2 * W]

                # ---- post ----
                u1 = postpool.tile([NROWS, W], BF, tag="u1")
                u2 = postpool.tile([NROWS, W], BF, tag="u2")
                u3 = postpool.tile([NROWS, W], BF, tag="u3")
                u4 = postpool.tile([NROWS, W], BF, tag="u4")
                u5 = postpool.tile([NROWS, W], BF, tag="u5")
                nc.vector.tensor_mul(u1[:], pk[:], iq[:])
                nc.vector.tensor_mul(u2[:], u1[:], ip[:])
                nc.vector.tensor_sub(u3[:], u2[:], lab)
                nc.vector.tensor_mul(u4[:], u3[:], ww)
                nc.vector.tensor_mul(u5[:], u4[:], u3[:])
                nc.vector.reduce_sum(acc[:, n:n + 1], u5[:],
                                     axis=mybir.AxisListType.X)

                g_prev = g
                godd_prev = godd

            nc.sync.dma_start(out[:], acc[:])
    _split_multi_waits(nc)
    return nc


def make_consts():
    eye = np.broadcast_to(np.eye(NSH, dtype=BF16), (C, NSH, NSH)).copy()
    eq = np.zeros((6, NSH, NROWS), BF16)
    for r in range(RPB):
        for m in range(NSH):
            eq[0, r, NSH * r + m] = 1
    for m, (dy, dx) in enumerate(SHIFTS):
        for r in range(RPB):
            eq[1 + (dx + 2), r + dy, NSH * r + m] = 1
    return eye, np.ascontiguousarray(eq.transpose(1, 0, 2))


def host_prep(er_input, seg_label, gt_boundary_seg, nblk=19):
    B, _, H, Wd_ = er_input.shape
    f32 = np.float32
    gb = np.where(gt_boundary_seg == 255, 0, gt_boundary_seg)
    slc = np.where(seg_label == 255, 0, seg_label)
    gt_b1 = gb * slc[:, 1]
    boundary = gt_b1 > 0
    iy = np.arange(H)
    ix = np.arange(Wd_)
    interior = (((iy >= 2) & (iy <= H - 3))[:, None]
                & ((ix >= 2) & (ix <= Wd_ - 3))[None, :])
    sel = boundary & interior
    cnt = sel.sum(axis=(1, 2)).astype(f32)
    valid = boundary.sum(axis=(1, 2)) >= 1
    n_valid = valid.astype(f32).sum()

    seg_f = seg_label.astype(f32)
    lab_stack = np.empty((NSH, B, H, Wd_), f32)
    w_stack = np.empty((NSH, B, H, Wd_), f32)
    sel_f = sel.astype(f32)
    for m, (dy, dx) in enumerate(SHIFTS):
        rolled = np.roll(seg_f, (-dy, -dx), axis=(2, 3))
        lab_stack[m] = (seg_f * rolled).sum(axis=1)
        sh = np.zeros_like(sel_f)
        ys0, ys1 = max(0, -dy), min(H, H - dy)
        xs0, xs1 = max(0, -dx), min(Wd_, Wd_ - dx)
        sh[:, ys0:ys1, xs0:xs1] = sel_f[:, ys0 + dy:ys1 + dy,
                                        xs0 + dx:xs1 + dx]
        w_stack[m] = sel_f + sh

    eye, eq = make_consts()
    slab_rows = 4 + RPB * nblk
    per_core = []
    for k in range(8):
        b, h = k // 2, k % 2
        r0 = 0 if h == 0 else 190
        xs = np.ascontiguousarray(
            er_input[b, :, r0:r0 + slab_rows, :].reshape(C, -1)).astype(f32)
        rows = r0 + 2 + np.arange(RPB * nblk)
        labc = lab_stack[:, b, rows, :].reshape(NSH, nblk, RPB, Wd_)
        wc = w_stack[:, b, rows, :].reshape(NSH, nblk, RPB, Wd_)
        labc = labc.transpose(1, 2, 0, 3).reshape(nblk, NROWS, Wd_)
        wc = wc.transpose(1, 2, 0, 3).reshape(nblk, NROWS, Wd_)
        lw = np.stack([labc, wc], axis=2).astype(BF16).reshape(
            nblk, NROWS, 2 * Wd_)
        per_core.append({"x": xs, "labw": lw, "eye": eye, "eq": eq})
    return per_core, dict(cnt=cnt, valid=valid, n_valid=n_valid)


def finish(core_sums, meta):
    f32 = np.float32
    cnt, valid, n_valid = meta["cnt"], meta["valid"], meta["n_valid"]
    total = f32(0.0)
    for b in range(4):
        sb = f32(core_sums[2 * b] + core_sums[2 * b + 1])
        loss_b = sb / max(cnt[b], f32(1.0)) / f32(24.0)
        if valid[b]:
            total = total + loss_b
    total = total / max(n_valid, f32(1.0))
    if np.isnan(total):
        total = f32(0.0)
    return np.float32(total)


_NC_CACHE = {}


def kernel(er_input, seg_label, gt_boundary_seg):
    er_input = np.asarray(er_input)
    seg_label = np.asarray(seg_label)
    gt_boundary_seg = np.asarray(gt_boundary_seg)
    per_core, meta = host_prep(er_input, seg_label, gt_boundary_seg)
    if "nc" not in _NC_CACHE:
        _NC_CACHE["nc"] = build_nc()
    nc = _NC_CACHE["nc"]
    res = bass_utils.run_bass_kernel_spmd(nc, per_core,
                                          core_ids=list(range(8)))
    sums = [r["out"].astype(np.float64).sum() for r in res.results]
    return finish(sums, meta)



# revision 2
# speedup vs baseline: 1.0895x; 1.0895x over previous
"""Bass TRN2 kernel v2 for the boundary cosine-similarity context loss.

Per core (8 cores): batch b = k//2, row-half h = k%2; 190 produced rows in
19 blocks x 10 rows. Host pre-converts features to bf16.

Row mapping inside a block: row = 64*w + 5*s + r5 for wave w in {0,1}
(produced rows 5w..5w+4), slot s in 0..11 (shift index SHIFTS[s]), r5 in
0..4. Rows 60..63 and 124..127 are dead (never used by host).

Per block n (y0 = 2+10n, win0 = y0*W):
  g    [C, 4616] bf16  <- x[:, win0   : win0+4616]   (direct HBM DMA)
  godd [C, 4612] bf16  <- x[:, win0+1 : win0+4613]   (odd-offset copy)
  sq = g[:, :4608]^2 (ACT, prefetched one block early)
  norms: 12 eye12 matmuls -> n2 psum [12, 384]; Ln -> Exp(-.5) -> inv12;
         ACT copy n2 -> n2sb (for the square-trick correction)
  products (per wave, in0 = g[:, base:base+1920]):
    slots 2..11 on DVE as 4 grouped TensorTensor ops (stride-2 window
    grids, stride-0 broadcast of in0);
    slots 0,1 via square trick: a = g + g_shift (Pool), t = a^2 (ACT);
    pd accumulates S - n2p - n2q = 2*dot via -1 h-matmuls (host folds
    lab *= 2, ww /= 4 for these slots).
  dots: per wave, 2 psum groups of 32 rows (eye32 one-hot matmuls,
        tile_position wants 32-aligned output bases).
  ip/iq: eq selection matmuls from inv12 -> psum.
  post (deferred one block; ACT copies + Pool math):
    pk=copy(pd); ipsb=copy(ipp); iqsb=copy(iqp); z = pk*ipsb*iqsb;
    v = z - lab; w1 = v*ww; w2 = w1*v; acc[:, n] = reduce_sum(w2).
Host folds 1/cnt, 1/24, valid, 1/n_valid.
"""

import numpy as np
import ml_dtypes

import concourse.bass as bass
import concourse.mybir as mybir
from concourse.tile import TileContext
from concourse.vector_clock import ScopedClock
from concourse import bass_utils

BF16 = ml_dtypes.bfloat16
F32 = mybir.dt.float32
BF = mybir.dt.bfloat16

W = 384
C = 128
SHIFTS = [(0, 1), (0, 2),
          (1, -2), (1, -1), (1, 0), (1, 1), (1, 2),
          (2, -2), (2, -1), (2, 0), (2, 1), (2, 2)]
OFFS = [dy * W + dx for dy, dx in SHIFTS]
NSH = 12
RPB = 10
NROWS = 128            # padded rows: 64*w + 5*s + r5; dead 60..63/124..127
SQSLOTS = (0, 1)       # slots via the square trick (dy=0)
# merged DVE products, one op per (parity): windows form a [dy, dx] grid
# with strides (384, 2); out slots stride (5, 2) from the first slot.
# evens: slots {2,4,6, 7,9,11} offs {382,384,386, 766,768,770} from g
# odds:  slots {3,5, 8,10} offs {383,385, 767,769} from godd (-1)
DVE_GROUPS = [(2, 3, "g", 382), (3, 2, "godd", 382)]
GLEN = 4616
ODLEN = 4612
NBLK = 19
NPIX = 194 * W
XPAD = 16


def _patch_tile_drain():
    if getattr(TileContext, "_drain_patched", False):
        return

    def _drain_and_barrier(self, tick_clock, wait_clock):
        drain_inst = self.nc.sync.drain()
        wait_clock.add_sem_waits(
            drain_inst.ins, ScopedClock({None: tick_clock.global_clock}))
        si = drain_inst.ins.sync_info
        if si is not None and si.on_wait and len(si.on_wait) > 1:
            waits = list(si.on_wait)
            drain_inst.ins.sync_info = mybir.SyncInfo(
                on_wait=[waits[-1]], on_update=list(si.on_update or []))
            for w in waits[:-1]:
                nop = self.nc.sync.nop(nofuse=True)
                nop.ins.sync_info = mybir.SyncInfo(on_wait=[w], on_update=[])
        self.nc.all_engine_barrier()
        popped = self.nc._tile_sem_poison_stack.pop()
        assert popped is self._sem_poison
        self.nc.clear_and_free_semaphores(list(self.sems.allocated().values()))
        self.nc.all_engine_barrier()

    TileContext._drain_and_barrier = _drain_and_barrier
    TileContext._drain_patched = True


_WSPLIT_N = [0]


def _split_multi_waits(nc, max_waits=1):
    """This container's walrus rejects instructions with more than one sync
    wait; hoist excess waits onto same-engine NOPs inserted just before."""
    for fn in nc.m.functions:
        for blk in fn.blocks:
            insts = blk.instructions
            out = []
            for inst in insts:
                si = inst.sync_info
                if si is not None and si.on_wait and len(si.on_wait) > max_waits:
                    waits = list(si.on_wait)
                    keep = waits[-max_waits:]
                    for w in waits[:-max_waits]:
                        _WSPLIT_N[0] += 1
                        nop = mybir.InstNoOp(
                            name=f"wsplit_{_WSPLIT_N[0]}", ins=[], outs=[])
                        nop.engine = inst.engine
                        nop.sync_info = mybir.SyncInfo(on_wait=[w],
                                                       on_update=[])
                        out.append(nop)
                    inst.sync_info = mybir.SyncInfo(
                        on_wait=keep, on_update=list(si.on_update or []))
                out.append(inst)
            blk.instructions = out


def _ap3(t, p_ap, off, d1, n1, d2, n2):
    return bass.AP(t.tensor, t.offset + off, [p_ap, [d1, n1], [d2, n2]])


def build_nc(nblk=NBLK, repeat=1):
    _patch_tile_drain()
    nc = bass.Bass()
    x = nc.dram_tensor("x", [C, NPIX + XPAD], BF, kind="ExternalInput")
    labw = nc.dram_tensor("labw", [nblk, NROWS, 2 * W], BF,
                          kind="ExternalInput")
    eye12 = nc.dram_tensor("eye12", [C, NSH, NSH], BF, kind="ExternalInput")
    eye64 = nc.dram_tensor("eye64", [C, 64, 64], BF, kind="ExternalInput")
    eq = nc.dram_tensor("eq", [NSH, 6, NROWS], BF, kind="ExternalInput")
    hwt = nc.dram_tensor("hwt", [NSH, 6, 64], BF, kind="ExternalInput")
    out = nc.dram_tensor("out", [NROWS, nblk], F32, kind="ExternalOutput")

    with TileContext(nc) as tc:
        with (tc.tile_pool(name="const", bufs=1) as cpool,
              tc.tile_pool(name="gbuf", bufs=2) as gpool,
              tc.tile_pool(name="sqp", bufs=2) as sqpool,
              tc.tile_pool(name="tp", bufs=2) as tpool,
              tc.tile_pool(name="apl", bufs=4) as apool,
              tc.tile_pool(name="inv", bufs=2) as ipool,
              tc.tile_pool(name="post", bufs=2) as postpool,
              tc.tile_pool(name="npsum", bufs=2, space="PSUM") as npsum,
              tc.tile_pool(name="dpsum", bufs=2, space="PSUM") as dpsum,
              tc.tile_pool(name="ppsum", bufs=2, space="PSUM") as ppsum,
              tc.tile_pool(name="qpsum", bufs=2, space="PSUM") as qpsum):

            eye12_sb = cpool.tile([C, NSH, NSH], BF)
            nc.sync.dma_start(eye12_sb[:], eye12[:])
            eye64_sb = cpool.tile([C, 64, 64], BF)
            nc.sync.dma_start(eye64_sb[:], eye64[:])
            eq_sb = cpool.tile([NSH, 6, NROWS], BF)
            nc.sync.dma_start(eq_sb[:], eq[:])
            hw_sb = cpool.tile([NSH, 6, 64], BF)
            nc.sync.dma_start(hw_sb[:], hwt[:])
            acc = cpool.tile([NROWS, nblk], F32)

            def load_g(n):
                win0 = (2 + RPB * n) * W
                g = gpool.tile([C, GLEN], BF, tag="g", name=f"g{n}")
                nc.sync.dma_start(g[:], x[:, win0:win0 + GLEN])
                godd = gpool.tile([C, ODLEN], BF, tag="godd", name=f"go{n}")
                nc.sync.dma_start(godd[:], x[:, win0 + 1:win0 + 1 + ODLEN])
                return g, godd

            def square(g, n):
                sq = sqpool.tile([C, 12 * W], BF, tag="sq", name=f"sq{n}")
                nc.scalar.square(sq[:], g[:, 0:12 * W])
                return sq

            g, godd = load_g(0)
            sq = square(g, 0)
            prev = None
            pool_post = None
            pending_reduce = None

            total = nblk * repeat
            for it in range(total):
                n = it % nblk
                first = it <= 1

                # ---- ACT copies for the previous block (unblock Pool) ----
                if prev is not None:
                    pool_post = _emit_copies(nc, postpool, *prev)
                    prev = None

                # ---- Pool adds + DVE grouped products ----
                tw = []
                for w in range(2):
                    base = w * 5 * W
                    t = tpool.tile([C, NSH, 5 * W], BF, tag="t",
                                   name=f"t{it}_{w}")
                    tw.append(t)
                    for s0, nw, src, foff in DVE_GROUPS:
                        srct = g if src == "g" else godd
                        p_g = g[:].ap[0]
                        in0b = bass.AP(g.tensor, g.offset + base,
                                       [p_g, [0, 2], [0, nw], [1, 5 * W]])
                        in1 = bass.AP(srct.tensor, srct.offset + base + foff,
                                      [srct[:].ap[0], [384, 2], [2, nw],
                                       [1, 5 * W]])
                        ot = bass.AP(t.tensor, t.offset + s0 * 5 * W,
                                     [t[:].ap[0], [5 * 5 * W, 2],
                                      [2 * 5 * W, nw], [1, 5 * W]])
                        nc.vector.tensor_mul(ot, in0b, in1)
                    for s in SQSLOTS:
                        off = OFFS[s]
                        if off % 2 == 0:
                            in1 = g[:, base + off:base + off + 5 * W]
                        else:
                            in1 = godd[:, base + off - 1:
                                       base + off - 1 + 5 * W]
                        a = apool.tile([C, 5 * W], BF, tag="a",
                                       name=f"a{it}_{w}_{s}")
                        nc.gpsimd.tensor_add(a[:],
                                             g[:, base:base + 5 * W], in1)
                        nc.scalar.square(t[:, s, :], a[:])

                # ---- norms ----
                n2 = npsum.tile([NSH, W], F32, tag="n2", name=f"n2_{it}")
                for j in range(12):
                    nc.tensor.matmul(n2[:], eye12_sb[:, j, :],
                                     sq[:, j * W:(j + 1) * W],
                                     start=(j == 0), stop=(j == 11))
                n2sb = ipool.tile([NSH, W + 4], BF, tag="n2sb",
                                  name=f"n2sb{it}")
                nc.scalar.copy(n2sb[:, 2:W + 2], n2[:])
                lnt = ipool.tile([NSH, W], F32, tag="lnt", name=f"lnt{it}")
                nc.scalar.activation(lnt[:], n2[:],
                                     mybir.ActivationFunctionType.Ln)
                inv12 = ipool.tile([NSH, W + 4], BF, tag="inv12",
                                   name=f"inv{it}")
                nc.scalar.activation(inv12[:, 2:W + 2], lnt[:],
                                     mybir.ActivationFunctionType.Exp,
                                     scale=-0.5)
                if first:
                    nc.vector.memset(inv12[:, 0:2], 0.0)
                    nc.vector.memset(inv12[:, W + 2:W + 4], 0.0)
                    nc.vector.memset(n2sb[:, 0:2], 0.0)
                    nc.vector.memset(n2sb[:, W + 2:W + 4], 0.0)

                # ---- dot matmuls ----
                pd = dpsum.tile([NROWS, W], F32, tag="pd", name=f"pd{it}")
                for w in range(2):
                    t = tw[w]
                    base_row = 64 * w
                    rows = list(range(10, 60)) + list(range(0, 10))
                    for i, row in enumerate(rows):
                        s, r5 = row // 5, row % 5
                        nc.tensor.matmul(
                            pd[base_row:base_row + 64, :],
                            eye64_sb[:, row, :],
                            t[:, s, r5 * W:(r5 + 1) * W],
                            start=(i == 0), stop=False)
                    for i, (hj, dxo) in enumerate([(0, 0), (1, 1), (2, 2)]):
                        nc.tensor.matmul(
                            pd[base_row:base_row + 64, :],
                            hw_sb[:, 3 * w + hj, :],
                            n2sb[:, 2 + dxo:2 + dxo + W],
                            start=False, stop=(i == 2))

                # ---- ip / iq selection matmuls ----
                ipp = ppsum.tile([NROWS, W], F32, tag="ipp", name=f"ipp{it}")
                nc.tensor.matmul(ipp[:], eq_sb[:, 0, :],
                                 inv12[:, 2:W + 2], start=True, stop=True)
                iqp = qpsum.tile([NROWS, W], F32, tag="iqp", name=f"iqp{it}")
                for di in range(5):
                    nc.tensor.matmul(iqp[:], eq_sb[:, 1 + di, :],
                                     inv12[:, di:di + W],
                                     start=(di == 0), stop=(di == 4))

                # ---- prefetch next block's g + sq ----
                if it + 1 < total:
                    g, godd = load_g((it + 1) % nblk)
                    sq = square(g, it + 1)

                # ---- deferred reduce (block it-2), then pool chain (it-1) ----
                if pending_reduce is not None:
                    w2p, np_ = pending_reduce
                    nc.vector.reduce_sum(acc[:, np_:np_ + 1], w2p[:],
                                         axis=mybir.AxisListType.X)
                    pending_reduce = None
                if pool_post is not None:
                    pending_reduce = _emit_pool_chain(nc, cpool, *pool_post)
                    pool_post = None
                lw = postpool.tile([NROWS, 2 * W], BF, tag="lw",
                                   name=f"lw{it}")
                nc.sync.dma_start(lw[:], labw[n])
                prev = (pd, ipp, iqp, lw, n)

            pool_post = _emit_copies(nc, postpool, *prev)
            if pending_reduce is not None:
                w2p, np_ = pending_reduce
                nc.vector.reduce_sum(acc[:, np_:np_ + 1], w2p[:],
                                     axis=mybir.AxisListType.X)
            w2p, np_ = _emit_pool_chain(nc, cpool, *pool_post)
            nc.vector.reduce_sum(acc[:, np_:np_ + 1], w2p[:],
                                 axis=mybir.AxisListType.X)
            nc.sync.dma_start(out[:], acc[:])
    _split_multi_waits(nc)
    return nc


def _emit_copies(nc, postpool, pd, ipp, iqp, lw, n):
    pk = postpool.tile([NROWS, W], BF, tag="pk", name=f"pk{n}")
    nc.scalar.copy(pk[:], pd[:])
    ipsb = postpool.tile([NROWS, W], BF, tag="ipsb", name=f"ipsb{n}")
    nc.scalar.copy(ipsb[:], ipp[:])
    iqsb = postpool.tile([NROWS, W], BF, tag="iqsb", name=f"iqsb{n}")
    nc.scalar.copy(iqsb[:], iqp[:])
    return (pk, ipsb, iqsb, lw, n)


def _emit_pool_chain(nc, cpool, pk, ipsb, iqsb, lw, n):
    z2 = cpool.tile([NROWS, W], BF, tag="z2", bufs=2, name=f"z2_{n}")
    nc.gpsimd.tensor_mul(z2[:], pk[:], ipsb[:])
    z = cpool.tile([NROWS, W], BF, tag="z", bufs=2, name=f"z{n}")
    nc.gpsimd.tensor_mul(z[:], z2[:], iqsb[:])
    v = cpool.tile([NROWS, W], BF, tag="v", bufs=2, name=f"v{n}")
    nc.gpsimd.tensor_sub(v[:], z[:], lw[:, 0:W])
    w1 = cpool.tile([NROWS, W], BF, tag="w1", bufs=2, name=f"w1_{n}")
    nc.gpsimd.tensor_mul(w1[:], v[:], lw[:, W:2 * W])
    w2 = cpool.tile([NROWS, W], BF, tag="w2", bufs=2, name=f"w2_{n}")
    nc.gpsimd.tensor_mul(w2[:], w1[:], v[:])
    return (w2, n)


def make_consts():
    eye12 = np.broadcast_to(np.eye(NSH, dtype=BF16), (C, NSH, NSH)).copy()
    eye64 = np.broadcast_to(np.eye(64, dtype=BF16), (C, 64, 64)).copy()
    # eq[d, j, row]: selection weights, row = 64w + 5s + r5, j = window row.
    eq = np.zeros((6, NSH, NROWS), BF16)
    for w in range(2):
        for s in range(NSH):
            dy, dx = SHIFTS[s]
            for r5 in range(5):
                row = 64 * w + 5 * s + r5
                j = 5 * w + r5
                eq[0, j, row] = 1                    # ip: inv at produced row
                eq[1 + (dx + 2), j + dy, row] = 1    # iq: inv at partner
    # hwm[w*3 + k, j, col]: -1 one-hots, group rows 64w..64w+63, col=5s+r5.
    hwm = np.zeros((6, NSH, 64), BF16)
    for w in range(2):
        for si, s in enumerate(SQSLOTS):
            dy, dx = SHIFTS[s]
            for r5 in range(5):
                col = 5 * s + r5
                j = 5 * w + r5
                hwm[3 * w + 0, j, col] = -1            # -n2 at produced pixel
                hwm[3 * w + 1 + si, j + dy, col] = -1  # -n2 at partner
    return eye12, eye64, np.ascontiguousarray(eq.transpose(1, 0, 2)), \
        np.ascontiguousarray(hwm.transpose(1, 0, 2))


def host_prep(er_input, seg_label, gt_boundary_seg, nblk=NBLK):
    B, _, H, Wd = er_input.shape
    f32 = np.float32
    gb = np.where(gt_boundary_seg == 255, 0, gt_boundary_seg)
    slc = np.where(seg_label == 255, 0, seg_label)
    gt_b1 = gb * slc[:, 1]
    boundary = gt_b1 > 0
    iy = np.arange(H)
    ix = np.arange(Wd)
    interior = (((iy >= 2) & (iy <= H - 3))[:, None]
                & ((ix >= 2) & (ix <= Wd - 3))[None, :])
    sel = boundary & interior
    cnt = sel.sum(axis=(1, 2)).astype(f32)
    valid = boundary.sum(axis=(1, 2)) >= 1
    n_valid = valid.astype(f32).sum()

    seg_f = seg_label.astype(f32)
    lab_stack = np.empty((NSH, B, H, Wd), f32)
    w_stack = np.empty((NSH, B, H, Wd), f32)
    sel_f = sel.astype(f32)
    for m, (dy, dx) in enumerate(SHIFTS):
        rolled = np.roll(seg_f, (-dy, -dx), axis=(2, 3))
        lab_stack[m] = (seg_f * rolled).sum(axis=1)
        sh = np.zeros_like(sel_f)
        ys0, ys1 = max(0, -dy), min(H, H - dy)
        xs0, xs1 = max(0, -dx), min(Wd, Wd - dx)
        sh[:, ys0:ys1, xs0:xs1] = sel_f[:, ys0 + dy:ys1 + dy,
                                        xs0 + dx:xs1 + dx]
        w_stack[m] = sel_f + sh
    # square-trick slots: kernel computes 2*dot -> z = 2*cos
    for s in SQSLOTS:
        lab_stack[s] *= 2.0
        w_stack[s] *= 0.25

    eye12, eye64, eqm, hwm = make_consts()
    per_core = []
    for k in range(8):
        b, h = k // 2, k % 2
        r0 = 0 if h == 0 else 190
        xs = np.zeros((C, NPIX + XPAD), BF16)
        xs[:, :NPIX] = er_input[b, :, r0:r0 + 194, :].reshape(C, -1)
        # lwc[n, 64w+5s+r5] <- (lab, ww)[s] at global row r0+2+10n+5w+r5
        gr = (r0 + 2 + RPB * np.arange(nblk)[:, None, None]
              + 5 * np.arange(2)[None, :, None]
              + np.arange(5)[None, None, :])          # [nblk, 2, 5]
        lab_g = lab_stack[:, b][:, gr]                # [NSH, nblk, 2, 5, W]
        ww_g = w_stack[:, b][:, gr]
        lwc = np.zeros((nblk, NROWS, 2 * Wd), BF16)
        rowidx = (64 * np.arange(2)[:, None, None]
                  + 5 * np.arange(NSH)[None, :, None]
                  + np.arange(5)[None, None, :]).reshape(-1)  # [2*12*5]
        lab_p = lab_g.transpose(1, 2, 0, 3, 4).reshape(nblk, 120, Wd)
        ww_p = ww_g.transpose(1, 2, 0, 3, 4).reshape(nblk, 120, Wd)
        lwc[:, rowidx, 0:Wd] = lab_p
        lwc[:, rowidx, Wd:2 * Wd] = ww_p
        per_core.append({"x": xs, "labw": lwc, "eye12": eye12,
                         "eye64": eye64, "eq": eqm, "hwt": hwm})
    return per_core, dict(cnt=cnt, valid=valid, n_valid=n_valid)


REAL_ROWS = np.array([r for r in range(NROWS) if (r % 64) < 60])


def finish(core_sums, meta):
    f32 = np.float32
    cnt, valid, n_valid = meta["cnt"], meta["valid"], meta["n_valid"]
    total = f32(0.0)
    for b in range(4):
        sb = f32(core_sums[2 * b] + core_sums[2 * b + 1])
        loss_b = sb / max(cnt[b], f32(1.0)) / f32(24.0)
        if valid[b]:
            total = total + loss_b
    total = total / max(n_valid, f32(1.0))
    if np.isnan(total):
        total = f32(0.0)
    return np.float32(total)


_NC_CACHE = {}


def kernel(er_input, seg_label, gt_boundary_seg):
    er_input = np.asarray(er_input)
    seg_label = np.asarray(seg_label)
    gt_boundary_seg = np.asarray(gt_boundary_seg)
    per_core, meta = host_prep(er_input, seg_label, gt_boundary_seg)
    if "nc" not in _NC_CACHE:
        _NC_CACHE["nc"] = build_nc()
    nc = _NC_CACHE["nc"]
    res = bass_utils.run_bass_kernel_spmd(nc, per_core,
                                          core_ids=list(range(8)))
    sums = [r["out"][REAL_ROWS].astype(np.float64).sum()
            for r in res.results]
    return finish(sums, meta)
